# revision 38
# baseline (speedup 1.0000x reference)
"""Trainium2 Bass kernel for the Conservative45K CNN+QNN model.

Strategy (pure data parallelism, 8 cores, B=512 images each):
  - Host: build banded-Toeplitz matrices for the three convs, pool and fc
    matrices, the composed 256x256 quantum operator U, and MLP weights.
    Conv operands are quantized to fp8-e4m3 (TRN flavor, max +-240) and
    packed as DoubleRow pairs [128, 2, 128]; x is shipped as fp8 in a
    zero-padded [128, 34, B] row-pair layout so every conv row is exactly
    two DoubleRow matmuls with no edge cases.
  - Device: conv1/2/3 as fp8 DoubleRow PE matmuls (2 K-subtiles per
    stream), relu+quantize on ACT/DVE alternating, 8x8 avg-pool as fp8
    DoubleRow matmuls over an fp8 h3 tile, then a fp16 head: fc -> relu,
    quantum <Z0> via y = U @ feats and sum of z-weighted y^2, and the
    1-200-150-100-50-1 MLP with sigmoid (f32 out).
  - PE warmup matmuls run during the DMA lead-in so the HAM clock gate
    reaches 2.4 GHz before conv1 starts.
"""

import sys

sys.path.insert(0, "/opt/trn_rl_repo")

import numpy as np

N_CORES = 8
B_TOTAL = 4096
B = B_TOTAL // N_CORES  # images per core (= matmul N)

# ---------------------------------------------------------------------------
# Host-side weight preprocessing (numpy only)
# ---------------------------------------------------------------------------


def _build_U(qw):
    """Compose the 8-qubit circuit (7x [RY layer + CNOT chain]) into a
    single 256x256 real matrix U (float64)."""
    NQ = 8
    psi = np.eye(256, dtype=np.float64).reshape((256,) + (2,) * NQ)
    for l in range(7):
        for q in range(NQ):
            th = float(qw[l, q]) / 2.0
            c, s = np.cos(th), np.sin(th)
            M = np.array([[c, -s], [s, c]], dtype=np.float64)
            a = q + 1
            pm = np.moveaxis(psi, a, 1)
            out = np.einsum("ij,bj...->bi...", M, pm)
            psi = np.moveaxis(out, 1, a)
        for q in range(NQ - 1):
            ac, at = q + 1, q + 2
            pm = np.moveaxis(psi, (ac, at), (1, 2))
            top = pm[:, 0]
            bot = np.flip(pm[:, 1], axis=1)
            pm = np.stack([top, bot], axis=1)
            psi = np.moveaxis(pm, (1, 2), (ac, at))
    rows = psi.reshape(256, 256)  # row i = U @ e_i = U[:, i]
    return rows.T  # U[j, k]


def _conv1_mats(w1):
    """Three [128,128] mats: lhsT[p, m] maps an input row-pair (2 rows of
    64 px) to one conv1 output row (32 x_out x 4 co).
    Mat A = pair y-1 (ky 0,1), B = pair y (ky 2,3), C = pair y+1 (ky 4)."""
    mats = np.zeros((3, 128, 128), dtype=np.float64)
    for p in range(128):
        sub, col = p // 64, p % 64
        for m in range(128):
            x_out, co = m // 4, m % 4
            kx = col - 2 * x_out + 2
            if not (0 <= kx < 5):
                continue
            for i, ky in enumerate((sub, 2 + sub, 4 if sub == 0 else -1)):
                if 0 <= ky < 5:
                    mats[i, p, m] = w1[co, 0, ky, kx]
    return mats


def _conv2_mats(w2):
    """Three [128,128] mats: input row q=(x_in*4+ci), output m=(x_out*8+co).
    Mat ky uses h1 row 2y'-1+ky."""
    mats = np.zeros((3, 128, 128), dtype=np.float64)
    for p in range(128):
        x_in, ci = p // 4, p % 4
        for m in range(128):
            x_out, co = m // 8, m % 8
            kx = x_in - 2 * x_out + 1
            if 0 <= kx < 3:
                for ky in range(3):
                    mats[ky, p, m] = w2[co, ci, ky, kx]
    return mats


def _conv3_mats(w3):
    """mats[ky][half] [128,128]: input row q=(x_in*8+ci), output
    m=(x_out*8+co_w), co = half*8+co_w. Uses h2 row y''-1+ky."""
    mats = np.zeros((3, 2, 128, 128), dtype=np.float64)
    for p in range(128):
        x_in, ci = p // 8, p % 8
        for m in range(128):
            x_out, co_w = m // 8, m % 8
            kx = x_in - x_out + 1
            if 0 <= kx < 3:
                for ky in range(3):
                    for half in range(2):
                        mats[ky, half, p, m] = w3[half * 8 + co_w, ci, ky, kx]
    return mats


def _pool_mats():
    """pm[yb] [128,32]: h3-row partition p=(x_in*8+co_w) of a row in block
    yb -> col m=(yb*16 + (x_in//8)*8 + co_w), entry 1/64."""
    pm = np.zeros((2, 128, 32), dtype=np.float64)
    for yb in range(2):
        for p in range(128):
            x_in, co_w = p // 8, p % 8
            pm[yb, p, yb * 16 + (x_in // 8) * 8 + co_w] = 1.0 / 64.0
    return pm


def _fc_mat(wf):
    """wfT [64, 256]: pooled partition p = half*32 + yb*16 + xb*8 + co_w
    maps to reference pooled index j = co*4 + yb*2 + xb, co = half*8+co_w."""
    wfT = np.zeros((64, 256), dtype=np.float64)
    for p in range(64):
        half, rem = p // 32, p % 32
        yb, xb, co_w = rem // 16, (rem % 16) // 8, rem % 8
        j = (half * 8 + co_w) * 4 + yb * 2 + xb
        wfT[p, :] = wf[:, j]
    return wfT


# fp8 blob: DoubleRow pair regions, each [128, 2*M] = (slot0 | slot1)
_BLOB8_SPECS = [
    ("c1A", 256), ("c1B", 256), ("c2A", 256), ("c2B", 256),
    ("c3A0", 256), ("c3B0", 256), ("c3A1", 256), ("c3B1", 256),
    ("plAB", 64),
]

N_KNOT_CAP = 126

_BLOB16_SPECS = [
    ("wfh0", (32, 256)), ("wfh1", (32, 256)),
    ("ut00", (128, 128)), ("ut01", (128, 128)),
    ("ut10", (128, 128)), ("ut11", (128, 128)),
    ("ones", (128, 1)), ("negones", (128, 1)),
    ("pwl_ones", (1, N_KNOT_CAP)),
    ("pwl_a", (N_KNOT_CAP, 1)), ("pwl_c1", (1, 1)),
]

_BLOBF_SPECS = [
    ("bias1", (128, 1)), ("bias2", (128, 1)),
    ("bias3h0", (128, 1)), ("bias3h1", (128, 1)),
    ("bf0", (128, 1)), ("bf1", (128, 1)),
    ("pwl_negt", (N_KNOT_CAP, 1)), ("pwl_c0", (1, 1)),
]


def _pwl_from_mlp(inputs):
    """Collapse the 1-200-150-100-50-1 relu MLP (a scalar piecewise-linear
    map q -> logit) into c0 + c1*q + sum_j a_j*relu(q - t_j)."""
    W = [np.asarray(inputs[f"wc{i}"], np.float64) for i in range(1, 6)]
    Bs = [np.asarray(inputs[f"bc{i}"], np.float64) for i in range(1, 6)]

    def g(q):
        h = q[None, :]
        for i in range(4):
            h = np.maximum(W[i] @ h + Bs[i][:, None], 0)
        return (W[4] @ h + Bs[4][:, None])[0]

    N = 1 << 20
    x = np.linspace(-1.05, 1.05, N)
    y = g(x)
    s = np.diff(y) / np.diff(x)
    ds = np.diff(s)
    knots = np.where(np.abs(ds) > 1e-9)[0]
    groups = (
        np.split(knots, np.where(np.diff(knots) > 1)[0] + 1) if len(knots) else []
    )
    t = np.array([x[g_[0] + 1] for g_ in groups])
    a = np.array([ds[g_].sum() for g_ in groups])
    c1 = s[0]
    c0 = y[0] - c1 * x[0]
    return t, a, c0, c1


def _layout(specs):
    offs, off = {}, 0
    for nm, w in specs:
        width = w if isinstance(w, int) else w[1]
        offs[nm] = (off, width)
        off += width
    return offs, off


def _host_prep(inputs):
    """Build xq per-core slices + the three weight blobs."""
    import ml_dtypes

    E4 = ml_dtypes.float8_e4m3  # TRN FP8_EXP4 flavor (max +-240)

    w1 = np.asarray(inputs["w1"], np.float64)
    w2 = np.asarray(inputs["w2"], np.float64)
    w3 = np.asarray(inputs["w3"], np.float64)
    wf = np.asarray(inputs["wf"], np.float64)
    qw = np.asarray(inputs["qw"], np.float64)

    c1 = _conv1_mats(w1)
    c2 = _conv2_mats(w2)
    c3 = _conv3_mats(w3)
    pm = _pool_mats()
    Z = np.zeros((128, 128))

    pairs = {
        "c1A": (c1[0], c1[1]), "c1B": (Z, c1[2]),
        "c2A": (c2[0], c2[1]), "c2B": (Z, c2[2]),
        "c3A0": (c3[0, 0], c3[1, 0]), "c3B0": (Z, c3[2, 0]),
        "c3A1": (c3[0, 1], c3[1, 1]), "c3B1": (Z, c3[2, 1]),
        "plAB": (pm[0], pm[1]),
    }
    offs8, w8 = _layout(_BLOB8_SPECS)
    blob8 = np.zeros((128, w8), dtype=E4)
    for nm, (a, b) in pairs.items():
        off, width = offs8[nm]
        cat = np.concatenate([a, b], axis=1).astype(np.float32)
        blob8[:, off : off + width] = np.clip(cat, -240, 240).astype(E4)

    mats16 = {}
    wfT = _fc_mat(wf)
    mats16["wfh0"], mats16["wfh1"] = wfT[:32], wfT[32:]
    U = _build_U(qw)
    UT = U.T
    for kc in range(2):
        for mh in range(2):
            mats16[f"ut{kc}{mh}"] = UT[kc * 128 : (kc + 1) * 128,
                                       mh * 128 : (mh + 1) * 128]
    t, a, c0, c1 = _pwl_from_mlp(inputs)
    nk = len(t)
    assert nk <= N_KNOT_CAP, f"PWL knot count {nk} exceeds cap"
    pa = np.zeros((N_KNOT_CAP, 1))
    pa[:nk, 0] = a
    pt = np.zeros((N_KNOT_CAP, 1))
    pt[:nk, 0] = -t
    mats16["ones"] = np.ones((128, 1))
    mats16["negones"] = -np.ones((128, 1))
    mats16["pwl_ones"] = np.ones((1, N_KNOT_CAP))
    mats16["pwl_a"] = pa
    mats16["pwl_c1"] = np.array([[c1]])

    offs16, w16 = _layout(_BLOB16_SPECS)
    blob16 = np.zeros((128, w16), dtype=np.float16)
    for nm, shape in _BLOB16_SPECS:
        off, width = offs16[nm]
        a = mats16[nm]
        assert a.shape == shape, (nm, a.shape, shape)
        blob16[: shape[0], off : off + width] = a.astype(np.float16)

    idx = np.arange(128)
    matsf = {
        "bias1": np.asarray(inputs["b1"], np.float64)[idx % 4].reshape(128, 1),
        "bias2": np.asarray(inputs["b2"], np.float64)[idx % 8].reshape(128, 1),
        "bias3h0": np.asarray(inputs["b3"], np.float64)[idx % 8].reshape(128, 1),
        "bias3h1": np.asarray(inputs["b3"], np.float64)[8 + idx % 8].reshape(128, 1),
        "bf0": np.asarray(inputs["bf"], np.float64)[:128].reshape(128, 1),
        "bf1": np.asarray(inputs["bf"], np.float64)[128:].reshape(128, 1),
        "pwl_negt": pt,
        "pwl_c0": np.array([[c0]]),
    }
    offsf, wfw = _layout(_BLOBF_SPECS)
    blobf = np.zeros((128, wfw), dtype=np.float32)
    for nm, shape in _BLOBF_SPECS:
        off, width = offsf[nm]
        blobf[: shape[0], off : off + width] = matsf[nm].astype(np.float32)

    # x: [128, 34, B] per core, idx r+1 = row-pair r of xT, idx 0/33 zero
    x = np.asarray(inputs["x"], np.float32).reshape(B_TOTAL, 64 * 64)
    xT = x.T  # [4096 px, B_TOTAL]
    x_slices = []
    for c in range(N_CORES):
        xs = xT[:, c * B : (c + 1) * B].reshape(32, 128, B).transpose(1, 0, 2)
        xq = np.zeros((128, 34, B), dtype=E4)
        xq[:, 1:33, :] = np.clip(xs, -240, 240).astype(E4)
        x_slices.append(np.ascontiguousarray(xq.reshape(128, 34 * B)))

    zero_b3 = not (np.any(matsf["bias3h0"]) or np.any(matsf["bias3h1"]))
    return x_slices, blob8, blob16, blobf, (zero_b3, nk)


# ---------------------------------------------------------------------------
# Device kernel
# ---------------------------------------------------------------------------

_COMPILED = {}


def _build_module(key):
    zero_b3, nk = key
    import concourse.bacc as bacc
    import concourse.tile as tile
    from concourse import mybir
    from contextlib import ExitStack

    f32 = mybir.dt.float32
    f32r = mybir.dt.float32r
    fp16 = mybir.dt.float16
    fp8 = mybir.dt.float8e4
    DRM = mybir.MatmulPerfMode.DoubleRow
    AF = mybir.ActivationFunctionType

    offs8, w8 = _layout(_BLOB8_SPECS)
    offs16, w16 = _layout(_BLOB16_SPECS)
    offsf, wfw = _layout(_BLOBF_SPECS)

    nc = bacc.Bacc("TRN2", debug=False, num_devices=N_CORES)
    xq_d = nc.dram_tensor("xq", [128, 34 * B], fp8, kind="ExternalInput").ap()
    b8_d = nc.dram_tensor("wq8", [128, w8], fp8, kind="ExternalInput").ap()
    b16_d = nc.dram_tensor("w16", [128, w16], fp16, kind="ExternalInput").ap()
    bf_d = nc.dram_tensor("wf32", [128, wfw], f32r, kind="ExternalInput").ap()
    out_d = nc.dram_tensor("out", [B], f32, kind="ExternalOutput").ap()

    with tile.TileContext(nc) as tc:
        stk = ExitStack()
        consts = stk.enter_context(tc.tile_pool(name="consts", bufs=1))
        b8_sb = consts.tile([128, w8], fp8, name="b8_sb", tag="b8")
        b16_sb = consts.tile([128, w16], fp16, name="b16_sb", tag="b16")
        bf_sb = consts.tile([128, wfw], f32r, name="bf_sb", tag="bf")
        xq_sb = consts.tile([128, 34 * B], fp8, name="xq_sb", tag="xq")
        h1t = consts.tile([128, 33 * B], fp8, name="h1t", tag="h1t")
        h2t = consts.tile([128, 18 * B], fp8, name="h2t", tag="h2t")
        h3t = consts.tile([128, 32 * B], fp8, name="h3t", tag="h3t")
        h3v = h3t[:].rearrange("p (r h b) -> p r h b", r=16, h=2)

        # weight blobs on the scalar-engine DMA queue; x on sync
        nc.scalar.dma_start(b8_sb[:], b8_d[:])
        nc.scalar.dma_start(bf_sb[:], bf_d[:])
        # first x chunk early so conv1 can start ASAP
        XCH = [(0, 6), (6, 13), (13, 20), (20, 27), (27, 34)]
        for a, b in XCH[:1]:
            nc.sync.dma_start(xq_sb[:, a * B : b * B], xq_d[:, a * B : b * B])
        nc.scalar.dma_start(b16_sb[:], b16_d[:])
        for a, b in XCH[1:]:
            nc.sync.dma_start(xq_sb[:, a * B : b * B], xq_d[:, a * B : b * B])

        def W8(nm):
            off, width = offs8[nm]
            return b8_sb[:, off : off + width]

        def W16(nm):
            off, width = offs16[nm]
            shape = dict(_BLOB16_SPECS)[nm]
            return b16_sb[0 : shape[0], off : off + width]

        def WF(nm):
            off, width = offsf[nm]
            shape = dict(_BLOBF_SPECS)[nm]
            return bf_sb[0 : shape[0], off : off + width]

        def DRW(nm):
            return W8(nm).rearrange("p (s m) -> p s m", s=2)

        def xsl(idx):
            return xq_sb[:, idx * B : (idx + 2) * B].rearrange(
                "p (s b) -> p s b", s=2
            )

        def h1sl(idx):
            return h1t[:, idx * B : (idx + 2) * B].rearrange(
                "p (s b) -> p s b", s=2
            )

        def h2sl(idx):
            return h2t[:, idx * B : (idx + 2) * B].rearrange(
                "p (s b) -> p s b", s=2
            )

        # ACT table warmup first: the sigmoid set also contains Relu, Square
        # and Copy, so one early load covers every ACT func in the kernel
        misc = stk.enter_context(tc.tile_pool(name="misc", bufs=1))
        warm = misc.tile([1, 2], f32, name="warm", tag="warm")
        nc.gpsimd.memset(warm[:], 0.0)
        warm2 = misc.tile([1, 2], f32, name="warm2", tag="warm2")
        nc.scalar.activation(warm2[:], warm[:], AF.Sigmoid)

        # zeroed pads
        nc.gpsimd.memset(h1t[:, 0:B], 0.0)
        nc.gpsimd.memset(h2t[:, 0:B], 0.0)
        nc.gpsimd.memset(h2t[:, 17 * B : 18 * B], 0.0)

        # single psum layout for the whole kernel: 2x [128,1536] (3 banks
        # each) + 2x [128,512] pool/head psums = exactly 8 banks, so no
        # mid-kernel pool-close barrier is ever needed
        cps = stk.enter_context(tc.tile_pool(name="cps", bufs=2, space="PSUM"))
        smps = stk.enter_context(tc.tile_pool(name="smps", bufs=2, space="PSUM"))

        relu_cnt = [0]

        def relu_wide(dst, src, bias_nm):
            """relu(+bias) alternating ACT/DVE; bias layout must be uniform
            across the free dim (per-partition)."""
            use_act = relu_cnt[0] % 2 == 0
            relu_cnt[0] += 1
            if bias_nm is None:
                if use_act:
                    nc.scalar.activation(dst, src, AF.Relu)
                else:
                    nc.vector.tensor_scalar_max(dst, src, 0.0)
            else:
                bias = WF(bias_nm).bitcast(f32)
                if use_act:
                    nc.scalar.activation(dst, src, AF.Relu, bias=bias)
                else:
                    nc.vector.tensor_scalar(
                        dst, src, bias, 0.0,
                        mybir.AluOpType.add, mybir.AluOpType.max,
                    )

        def relu_both(dst_tile, dst_off, ps, w, bias_nm):
            """relu a [128,w] psum group: ACT takes the first 1024 cols
            (2 psum regions), DVE the remainder (region-aligned so each
            piece gates on an earlier matmul stop)."""
            bias = WF(bias_nm).bitcast(f32)
            cut = min(1024, w - 512) if w > 512 else w
            nc.scalar.activation(
                dst_tile[:, dst_off : dst_off + cut], ps[:, 0:cut],
                AF.Relu, bias=bias,
            )
            if cut < w:
                nc.vector.tensor_scalar(
                    dst_tile[:, dst_off + cut : dst_off + w],
                    ps[:, cut:w], bias, 0.0,
                    mybir.AluOpType.add, mybir.AluOpType.max,
                )

        # ---- conv1: 3-row groups (2 DR matmuls per row) ----
        c1_groups = [(0 + 3 * g, 3) for g in range(10)] + [(30, 2)]
        for y0, nr in c1_groups:
            ps = cps.tile([128, 1536], f32, name=f"c1ps{y0}", tag="cps")
            for j in range(nr):
                nc.tensor.matmul(
                    ps[:, j * 512 : (j + 1) * 512], DRW("c1A"), xsl(y0 + j),
                    start=True, stop=False, perf_mode=DRM,
                )
            for j in range(nr):
                nc.tensor.matmul(
                    ps[:, j * 512 : (j + 1) * 512], DRW("c1B"), xsl(y0 + j + 1),
                    start=False, stop=True, perf_mode=DRM,
                )
            relu_both(h1t, (y0 + 1) * B, ps, nr * 512, "bias1")

        # ---- conv2: 3-row groups ----
        c2_groups = [(3 * g, 3) for g in range(5)] + [(15, 1)]
        for y0, nr in c2_groups:
            ps = cps.tile([128, 1536], f32, name=f"c2ps{y0}", tag="cps")
            for j in range(nr):
                nc.tensor.matmul(
                    ps[:, j * 512 : (j + 1) * 512], DRW("c2A"),
                    h1sl(2 * (y0 + j)),
                    start=True, stop=False, perf_mode=DRM,
                )
            for j in range(nr):
                nc.tensor.matmul(
                    ps[:, j * 512 : (j + 1) * 512], DRW("c2B"),
                    h1sl(2 * (y0 + j) + 1),
                    start=False, stop=True, perf_mode=DRM,
                )
            relu_both(h2t, (y0 + 1) * B, ps, nr * 512, "bias2")

        plt = [
            smps.tile([128, 512], f32, name=f"plt{h}", tag="sm") for h in range(2)
        ]

        def _pool_pair(k):
            # stream k contracts h3 rows (k, k+8): slot0 via pm_yb0 -> out
            # partitions 0:16, slot1 via pm_yb1 -> partitions 16:32
            for half in range(2):
                nc.tensor.matmul(
                    plt[half][0:32, :],
                    W8("plAB").rearrange("p (s m) -> p s m", s=2),
                    h3v[:, k : k + 9 : 8, half, :],
                    start=(k == 0), stop=(k == 7), perf_mode=DRM,
                )

        # ---- conv3 rows (2 DR per half) + interleaved pool DRs ----
        for y in range(16):
            ps = cps.tile([128, 1536], f32, name=f"c3ps{y}", tag="cps")
            for half, (nmA, nmB) in enumerate(
                (("c3A0", "c3B0"), ("c3A1", "c3B1"))
            ):
                o = ps[:, half * 512 : (half + 1) * 512]
                nc.tensor.matmul(o, DRW(nmA), h2sl(y), start=True, stop=False,
                                 perf_mode=DRM)
                nc.tensor.matmul(o, DRW(nmB), h2sl(y + 1), start=False,
                                 stop=True, perf_mode=DRM)
            if zero_b3:
                # region-aligned halves; alternate engines by row parity
                lo = h3t[:, y * 1024 : y * 1024 + 512]
                hi = h3t[:, y * 1024 + 512 : (y + 1) * 1024]
                if y % 2 == 0:
                    nc.scalar.activation(lo, ps[:, 0:512], AF.Relu)
                    nc.vector.tensor_scalar_max(hi, ps[:, 512:1024], 0.0)
                else:
                    nc.vector.tensor_scalar_max(lo, ps[:, 0:512], 0.0)
                    nc.scalar.activation(hi, ps[:, 512:1024], AF.Relu)
            else:
                relu_wide(h3v[:, y, 0, :], ps[:, 0:512], "bias3h0")
                relu_wide(h3v[:, y, 1, :], ps[:, 512:1024], "bias3h1")
            if y >= 10:
                _pool_pair(y - 10)
        _pool_pair(6)
        _pool_pair(7)

        # ---- head ----
        hsb = stk.enter_context(tc.tile_pool(name="hsb", bufs=2))

        pooled = []
        for half in range(2):
            t = hsb.tile([32, B], fp16, name=f"pooled{half}", tag="pooled")
            if half == 0:
                nc.scalar.activation(t[:], plt[half][0:32, :], AF.Copy)
            else:
                nc.vector.tensor_copy(t[:], plt[half][0:32, :])
            pooled.append(t)

        # fc: feats = relu(wf @ pooled + bf) as two [128,B] chunks
        feats, sqf = [], []
        for mh in range(2):
            ps = cps.tile([128, 1536], f32, name=f"fcps{mh}", tag="cps")
            for half in range(2):
                nc.tensor.matmul(
                    ps[:, 0:512],
                    W16(f"wfh{half}")[:, mh * 128 : (mh + 1) * 128],
                    pooled[half][:],
                    start=(half == 0), stop=(half == 1),
                )
            f = hsb.tile([128, B], fp16, name=f"feats{mh}", tag="feats")
            bias = WF(f"bf{mh}").bitcast(f32)
            if mh == 0:
                nc.scalar.activation(f[:], ps[:, 0:512], AF.Relu, bias=bias)
            else:
                nc.vector.tensor_scalar(
                    f[:], ps[:, 0:512], bias, 0.0,
                    mybir.AluOpType.add, mybir.AluOpType.max,
                )
            feats.append(f)
        # squares of feats for |feats|^2
        for mh in range(2):
            s = hsb.tile([128, B], fp16, name=f"sqf{mh}", tag="sqf")
            if mh == 0:
                nc.scalar.activation(s[:], feats[mh][:], AF.Square)
            else:
                nc.vector.tensor_mul(s[:], feats[mh][:], feats[mh][:])
            sqf.append(s)
        ssps = smps.tile([128, 512], f32, name="ssps", tag="sm")
        for mh in range(2):
            nc.tensor.matmul(
                ssps[0:1, :], W16("ones")[:, 0:1], sqf[mh][:],
                start=(mh == 0), stop=(mh == 1),
            )
        # y = U @ feats; zsum = sum z_j y_j^2
        zsps = smps.tile([128, 512], f32, name="zsps", tag="sm")
        for mh in range(2):
            ups = cps.tile([128, 1536], f32, name=f"ups{mh}", tag="cps")
            for kc in range(2):
                nc.tensor.matmul(
                    ups[:, 0:512], W16(f"ut{kc}{mh}"), feats[kc][:],
                    start=(kc == 0), stop=(kc == 1),
                )
            sqy = hsb.tile([128, B], fp16, name=f"sqy{mh}", tag="sqy")
            nc.scalar.activation(sqy[:], ups[:, 0:512], AF.Square)
            nc.tensor.matmul(
                zsps[0:1, :],
                (W16("ones") if mh == 0 else W16("negones"))[:, 0:1],
                sqy[:],
                start=(mh == 0), stop=(mh == 1),
            )
        # q = zsum / ss  (ss = |feats|^2 is O(1)-bounded away from 0 here)
        rss = hsb.tile([1, B], f32, name="rss", tag="qrow", bufs=6)
        rscr = hsb.tile([1, B], f32, name="rscr", tag="qrow", bufs=6)
        nc.vector.reciprocal_approx_accurate(rss[:], ssps[0:1, :], rscr[:])
        q = hsb.tile([1, B], fp16, name="q", tag="qrow", bufs=6)
        nc.vector.tensor_mul(q[:], zsps[0:1, :], rss[:])

        # MLP collapsed to its exact piecewise-linear form:
        # logit = c0 + c1*q + sum_j a_j * relu(q - t_j)
        o = hsb.tile([1, B], f32, name="sb_o", tag="mlpo")
        ops = smps.tile([128, 512], f32, name="ps_o", tag="sm")
        if nk == 0:
            nc.tensor.matmul(ops[0:1, :], W16("pwl_c1"), q[:],
                             start=True, stop=True)
        elif nk == 1:
            r = hsb.tile([1, B], fp16, name="pwl_r", tag="mlpa")
            nc.scalar.activation(
                r[:], q[:], AF.Relu, bias=WF("pwl_negt")[0:1, :].bitcast(f32)
            )
            nc.tensor.matmul(ops[0:1, :], W16("pwl_a")[0:1, :], r[:],
                             start=True, stop=False)
            nc.tensor.matmul(ops[0:1, :], W16("pwl_c1"), q[:],
                             start=False, stop=True)
        else:
            kn = smps.tile([128, 512], f32, name="ps_kn", tag="sm")
            nc.tensor.matmul(kn[0:nk, :], W16("pwl_ones")[:, 0:nk], q[:],
                             start=True, stop=True)
            r = hsb.tile([nk, B], fp16, name="pwl_r", tag="mlpa")
            nc.scalar.activation(
                r[:], kn[0:nk, :], AF.Relu,
                bias=WF("pwl_negt")[0:nk, :].bitcast(f32),
            )
            nc.tensor.matmul(ops[0:1, :], W16("pwl_a")[0:nk, :], r[:],
                             start=True, stop=False)
            nc.tensor.matmul(ops[0:1, :], W16("pwl_c1"), q[:],
                             start=False, stop=True)
        nc.scalar.activation(
            o[:], ops[0:1, :], AF.Sigmoid, bias=WF("pwl_c0")[0:1, :].bitcast(f32)
        )

        nc.sync.dma_start(out_d[:], o[:])
        stk.close()

    nc.compile()
    return nc


def kernel(**inputs):
    from concourse import bass_utils

    x_slices, blob8, blob16, blobf, key = _host_prep(inputs)
    if key not in _COMPILED:
        _COMPILED[key] = _build_module(key)
    nc = _COMPILED[key]

    in_maps = [
        {"xq": x_slices[c], "wq8": blob8, "w16": blob16, "wf32": blobf}
        for c in range(N_CORES)
    ]
    res = bass_utils.run_bass_kernel_spmd(nc, in_maps, list(range(N_CORES)))
    outs = [res.results[c]["out"].reshape(B, 1) for c in range(N_CORES)]
    return np.concatenate(outs, axis=0).astype(np.float32)


# revision 42
# speedup vs baseline: 1.0893x; 1.0893x over previous
"""Trainium2 Bass kernel for the Conservative45K CNN+QNN model.

Strategy (pure data parallelism, 8 cores, B=512 images each):
  - Host: build banded-Toeplitz matrices for the three convs, pool and fc
    matrices, the composed 256x256 quantum operator U, and MLP weights.
    Conv operands are quantized to fp8-e4m3 (TRN flavor, max +-240) and
    packed as DoubleRow pairs [128, 2, 128]; x is shipped as fp8 in a
    zero-padded [128, 34, B] row-pair layout so every conv row is exactly
    two DoubleRow matmuls with no edge cases.
  - Device: conv1/2/3 as fp8 DoubleRow PE matmuls (2 K-subtiles per
    stream), relu+quantize on ACT/DVE alternating, 8x8 avg-pool as fp8
    DoubleRow matmuls over an fp8 h3 tile, then a fp16 head: fc -> relu,
    quantum <Z0> via y = U @ feats and sum of z-weighted y^2, and the
    1-200-150-100-50-1 MLP with sigmoid (f32 out).
  - PE warmup matmuls run during the DMA lead-in so the HAM clock gate
    reaches 2.4 GHz before conv1 starts.
"""

import sys

sys.path.insert(0, "/opt/trn_rl_repo")

import numpy as np

N_CORES = 8
B_TOTAL = 4096
B = B_TOTAL // N_CORES  # images per core (= matmul N)

# ---------------------------------------------------------------------------
# Host-side weight preprocessing (numpy only)
# ---------------------------------------------------------------------------


def _build_U(qw):
    """Compose the 8-qubit circuit (7x [RY layer + CNOT chain]) into a
    single 256x256 real matrix U (float64)."""
    NQ = 8
    psi = np.eye(256, dtype=np.float64).reshape((256,) + (2,) * NQ)
    for l in range(7):
        for q in range(NQ):
            th = float(qw[l, q]) / 2.0
            c, s = np.cos(th), np.sin(th)
            M = np.array([[c, -s], [s, c]], dtype=np.float64)
            a = q + 1
            pm = np.moveaxis(psi, a, 1)
            out = np.einsum("ij,bj...->bi...", M, pm)
            psi = np.moveaxis(out, 1, a)
        for q in range(NQ - 1):
            ac, at = q + 1, q + 2
            pm = np.moveaxis(psi, (ac, at), (1, 2))
            top = pm[:, 0]
            bot = np.flip(pm[:, 1], axis=1)
            pm = np.stack([top, bot], axis=1)
            psi = np.moveaxis(pm, (1, 2), (ac, at))
    rows = psi.reshape(256, 256)  # row i = U @ e_i = U[:, i]
    return rows.T  # U[j, k]


def _conv1_mats(w1):
    """Three [128,128] mats: lhsT[p, m] maps an input row-pair (2 rows of
    64 px) to one conv1 output row (32 x_out x 4 co).
    Mat A = pair y-1 (ky 0,1), B = pair y (ky 2,3), C = pair y+1 (ky 4)."""
    mats = np.zeros((3, 128, 128), dtype=np.float64)
    for p in range(128):
        sub, col = p // 64, p % 64
        for m in range(128):
            x_out, co = m // 4, m % 4
            kx = col - 2 * x_out + 2
            if not (0 <= kx < 5):
                continue
            for i, ky in enumerate((sub, 2 + sub, 4 if sub == 0 else -1)):
                if 0 <= ky < 5:
                    mats[i, p, m] = w1[co, 0, ky, kx]
    return mats


def _conv2_mats(w2):
    """Three [128,128] mats: input row q=(x_in*4+ci), output m=(x_out*8+co).
    Mat ky uses h1 row 2y'-1+ky."""
    mats = np.zeros((3, 128, 128), dtype=np.float64)
    for p in range(128):
        x_in, ci = p // 4, p % 4
        for m in range(128):
            x_out, co = m // 8, m % 8
            kx = x_in - 2 * x_out + 1
            if 0 <= kx < 3:
                for ky in range(3):
                    mats[ky, p, m] = w2[co, ci, ky, kx]
    return mats


def _conv3_mats(w3):
    """mats[ky][half] [128,128]: input row q=(x_in*8+ci), output
    m=(x_out*8+co_w), co = half*8+co_w. Uses h2 row y''-1+ky."""
    mats = np.zeros((3, 2, 128, 128), dtype=np.float64)
    for p in range(128):
        x_in, ci = p // 8, p % 8
        for m in range(128):
            x_out, co_w = m // 8, m % 8
            kx = x_in - x_out + 1
            if 0 <= kx < 3:
                for ky in range(3):
                    for half in range(2):
                        mats[ky, half, p, m] = w3[half * 8 + co_w, ci, ky, kx]
    return mats


def _pool_mats():
    """pm[yb] [128,32]: h3-row partition p=(x_in*8+co_w) of a row in block
    yb -> col m=(yb*16 + (x_in//8)*8 + co_w), entry 1/64."""
    pm = np.zeros((2, 128, 32), dtype=np.float64)
    for yb in range(2):
        for p in range(128):
            x_in, co_w = p // 8, p % 8
            pm[yb, p, yb * 16 + (x_in // 8) * 8 + co_w] = 1.0 / 64.0
    return pm


def _fc_mat(wf):
    """wfT [64, 256]: pooled partition p = half*32 + yb*16 + xb*8 + co_w
    maps to reference pooled index j = co*4 + yb*2 + xb, co = half*8+co_w."""
    wfT = np.zeros((64, 256), dtype=np.float64)
    for p in range(64):
        half, rem = p // 32, p % 32
        yb, xb, co_w = rem // 16, (rem % 16) // 8, rem % 8
        j = (half * 8 + co_w) * 4 + yb * 2 + xb
        wfT[p, :] = wf[:, j]
    return wfT


# fp8 blob: DoubleRow pair regions, each [128, 2*M] = (slot0 | slot1)
_BLOB8_SPECS = [
    ("c1A", 256), ("c1B", 256), ("c2A", 256), ("c2B", 256),
    ("c3A0", 256), ("c3B0", 256), ("c3A1", 256), ("c3B1", 256),
    ("plAB", 64),
]

N_KNOT_CAP = 126

_BLOB16_SPECS = [
    ("wfh0", (32, 256)), ("wfh1", (32, 256)),
    ("ut00", (128, 128)), ("ut01", (128, 128)),
    ("ut10", (128, 128)), ("ut11", (128, 128)),
    ("ones", (128, 1)), ("negones", (128, 1)),
    ("pwl_ones", (1, N_KNOT_CAP)),
    ("pwl_a", (N_KNOT_CAP, 1)), ("pwl_c1", (1, 1)),
]

_BLOBF_SPECS = [
    ("bias1", (128, 1)), ("bias2", (128, 1)),
    ("bias3h0", (128, 1)), ("bias3h1", (128, 1)),
    ("bf0", (128, 1)), ("bf1", (128, 1)),
    ("pwl_negt", (N_KNOT_CAP, 1)), ("pwl_c0", (1, 1)),
]


def _pwl_from_mlp(inputs):
    """Collapse the 1-200-150-100-50-1 relu MLP (a scalar piecewise-linear
    map q -> logit) into c0 + c1*q + sum_j a_j*relu(q - t_j)."""
    W = [np.asarray(inputs[f"wc{i}"], np.float64) for i in range(1, 6)]
    Bs = [np.asarray(inputs[f"bc{i}"], np.float64) for i in range(1, 6)]

    def g(q):
        h = q[None, :]
        for i in range(4):
            h = np.maximum(W[i] @ h + Bs[i][:, None], 0)
        return (W[4] @ h + Bs[4][:, None])[0]

    N = 1 << 20
    x = np.linspace(-1.05, 1.05, N)
    y = g(x)
    s = np.diff(y) / np.diff(x)
    ds = np.diff(s)
    knots = np.where(np.abs(ds) > 1e-9)[0]
    groups = (
        np.split(knots, np.where(np.diff(knots) > 1)[0] + 1) if len(knots) else []
    )
    t = np.array([x[g_[0] + 1] for g_ in groups])
    a = np.array([ds[g_].sum() for g_ in groups])
    c1 = s[0]
    c0 = y[0] - c1 * x[0]
    return t, a, c0, c1


def _layout(specs):
    offs, off = {}, 0
    for nm, w in specs:
        width = w if isinstance(w, int) else w[1]
        offs[nm] = (off, width)
        off += width
    return offs, off


def _host_prep(inputs):
    """Build xq per-core slices + the three weight blobs."""
    import ml_dtypes

    E4 = ml_dtypes.float8_e4m3  # TRN FP8_EXP4 flavor (max +-240)

    w1 = np.asarray(inputs["w1"], np.float64)
    w2 = np.asarray(inputs["w2"], np.float64)
    w3 = np.asarray(inputs["w3"], np.float64)
    wf = np.asarray(inputs["wf"], np.float64)
    qw = np.asarray(inputs["qw"], np.float64)

    c1 = _conv1_mats(w1)
    c2 = _conv2_mats(w2)
    c3 = _conv3_mats(w3)
    pm = _pool_mats()
    Z = np.zeros((128, 128))

    pairs = {
        "c1A": (c1[0], c1[1]), "c1B": (Z, c1[2]),
        "c2A": (c2[0], c2[1]), "c2B": (Z, c2[2]),
        "c3A0": (c3[0, 0], c3[1, 0]), "c3B0": (Z, c3[2, 0]),
        "c3A1": (c3[0, 1], c3[1, 1]), "c3B1": (Z, c3[2, 1]),
        "plAB": (pm[0], pm[1]),
    }
    offs8, w8 = _layout(_BLOB8_SPECS)
    blob8 = np.zeros((128, w8), dtype=E4)
    for nm, (a, b) in pairs.items():
        off, width = offs8[nm]
        cat = np.concatenate([a, b], axis=1).astype(np.float32)
        blob8[:, off : off + width] = np.clip(cat, -240, 240).astype(E4)

    mats16 = {}
    wfT = _fc_mat(wf)
    mats16["wfh0"], mats16["wfh1"] = wfT[:32], wfT[32:]
    U = _build_U(qw)
    UT = U.T
    for kc in range(2):
        for mh in range(2):
            mats16[f"ut{kc}{mh}"] = UT[kc * 128 : (kc + 1) * 128,
                                       mh * 128 : (mh + 1) * 128]
    t, a, c0, c1 = _pwl_from_mlp(inputs)
    nk = len(t)
    assert nk <= N_KNOT_CAP, f"PWL knot count {nk} exceeds cap"
    pa = np.zeros((N_KNOT_CAP, 1))
    pa[:nk, 0] = a
    pt = np.zeros((N_KNOT_CAP, 1))
    pt[:nk, 0] = -t
    mats16["ones"] = np.ones((128, 1))
    mats16["negones"] = -np.ones((128, 1))
    mats16["pwl_ones"] = np.ones((1, N_KNOT_CAP))
    mats16["pwl_a"] = pa
    mats16["pwl_c1"] = np.array([[c1]])

    offs16, w16 = _layout(_BLOB16_SPECS)
    blob16 = np.zeros((128, w16), dtype=np.float16)
    for nm, shape in _BLOB16_SPECS:
        off, width = offs16[nm]
        a = mats16[nm]
        assert a.shape == shape, (nm, a.shape, shape)
        blob16[: shape[0], off : off + width] = a.astype(np.float16)

    idx = np.arange(128)
    matsf = {
        "bias1": np.asarray(inputs["b1"], np.float64)[idx % 4].reshape(128, 1),
        "bias2": np.asarray(inputs["b2"], np.float64)[idx % 8].reshape(128, 1),
        "bias3h0": np.asarray(inputs["b3"], np.float64)[idx % 8].reshape(128, 1),
        "bias3h1": np.asarray(inputs["b3"], np.float64)[8 + idx % 8].reshape(128, 1),
        "bf0": np.asarray(inputs["bf"], np.float64)[:128].reshape(128, 1),
        "bf1": np.asarray(inputs["bf"], np.float64)[128:].reshape(128, 1),
        "pwl_negt": pt,
        "pwl_c0": np.array([[c0]]),
    }
    offsf, wfw = _layout(_BLOBF_SPECS)
    blobf = np.zeros((128, wfw), dtype=np.float32)
    for nm, shape in _BLOBF_SPECS:
        off, width = offsf[nm]
        blobf[: shape[0], off : off + width] = matsf[nm].astype(np.float32)

    # x: [128, 34, B] per core, idx r+1 = row-pair r of xT, idx 0/33 zero
    x = np.asarray(inputs["x"], np.float32).reshape(B_TOTAL, 64 * 64)
    xT = x.T  # [4096 px, B_TOTAL]
    x_slices = []
    for c in range(N_CORES):
        xs = xT[:, c * B : (c + 1) * B].reshape(32, 128, B).transpose(1, 0, 2)
        xq = np.zeros((128, 34, B), dtype=E4)
        xq[:, 1:33, :] = np.clip(xs, -240, 240).astype(E4)
        x_slices.append(np.ascontiguousarray(xq.reshape(128, 34 * B)))

    zero_b3 = not (np.any(matsf["bias3h0"]) or np.any(matsf["bias3h1"]))
    return x_slices, blob8, blob16, blobf, (zero_b3, nk)


# ---------------------------------------------------------------------------
# Device kernel
# ---------------------------------------------------------------------------

_COMPILED = {}


def _build_module(key):
    zero_b3, nk = key
    import concourse.bacc as bacc
    import concourse.tile as tile
    from concourse import mybir
    from contextlib import ExitStack

    f32 = mybir.dt.float32
    f32r = mybir.dt.float32r
    fp16 = mybir.dt.float16
    fp8 = mybir.dt.float8e4
    DRM = mybir.MatmulPerfMode.DoubleRow
    AF = mybir.ActivationFunctionType

    offs8, w8 = _layout(_BLOB8_SPECS)
    offs16, w16 = _layout(_BLOB16_SPECS)
    offsf, wfw = _layout(_BLOBF_SPECS)

    nc = bacc.Bacc("TRN2", debug=False, num_devices=N_CORES)
    xq_d = nc.dram_tensor("xq", [128, 34 * B], fp8, kind="ExternalInput").ap()
    b8_d = nc.dram_tensor("wq8", [128, w8], fp8, kind="ExternalInput").ap()
    b16_d = nc.dram_tensor("w16", [128, w16], fp16, kind="ExternalInput").ap()
    bf_d = nc.dram_tensor("wf32", [128, wfw], f32r, kind="ExternalInput").ap()
    out_d = nc.dram_tensor("out", [B], f32, kind="ExternalOutput").ap()

    with tile.TileContext(nc) as tc:
        stk = ExitStack()
        consts = stk.enter_context(tc.tile_pool(name="consts", bufs=1))
        b8_sb = consts.tile([128, w8], fp8, name="b8_sb", tag="b8")
        b16_sb = consts.tile([128, w16], fp16, name="b16_sb", tag="b16")
        bf_sb = consts.tile([128, wfw], f32r, name="bf_sb", tag="bf")
        xq_sb = consts.tile([128, 34 * B], fp8, name="xq_sb", tag="xq")
        h1t = consts.tile([128, 33 * B], fp8, name="h1t", tag="h1t")
        h2t = consts.tile([128, 18 * B], fp8, name="h2t", tag="h2t")
        h3t = consts.tile([128, 32 * B], fp8, name="h3t", tag="h3t")
        h3v = h3t[:].rearrange("p (r h b) -> p r h b", r=16, h=2)

        # weight blobs on the vector-engine DMA queue (keeps the scalar
        # queue free for ACT table loads); x on sync
        nc.gpsimd.dma_start(b8_sb[:], b8_d[:])
        nc.gpsimd.dma_start(bf_sb[:], bf_d[:])
        # first x chunk early so conv1 can start ASAP
        XCH = [(0, 6), (6, 13), (13, 20), (20, 27), (27, 34)]
        for a, b in XCH[:1]:
            nc.sync.dma_start(xq_sb[:, a * B : b * B], xq_d[:, a * B : b * B])
        nc.gpsimd.dma_start(b16_sb[:], b16_d[:])
        for a, b in XCH[1:]:
            nc.sync.dma_start(xq_sb[:, a * B : b * B], xq_d[:, a * B : b * B])

        def W8(nm):
            off, width = offs8[nm]
            return b8_sb[:, off : off + width]

        def W16(nm):
            off, width = offs16[nm]
            shape = dict(_BLOB16_SPECS)[nm]
            return b16_sb[0 : shape[0], off : off + width]

        def WF(nm):
            off, width = offsf[nm]
            shape = dict(_BLOBF_SPECS)[nm]
            return bf_sb[0 : shape[0], off : off + width]

        def DRW(nm):
            return W8(nm).rearrange("p (s m) -> p s m", s=2)

        def xsl(idx):
            return xq_sb[:, idx * B : (idx + 2) * B].rearrange(
                "p (s b) -> p s b", s=2
            )

        def h1sl(idx):
            return h1t[:, idx * B : (idx + 2) * B].rearrange(
                "p (s b) -> p s b", s=2
            )

        def h2sl(idx):
            return h2t[:, idx * B : (idx + 2) * B].rearrange(
                "p (s b) -> p s b", s=2
            )

        # ACT table warmup first: the sigmoid set also contains Relu, Square
        # and Copy, so one early load covers every ACT func in the kernel
        misc = stk.enter_context(tc.tile_pool(name="misc", bufs=1))
        warm = misc.tile([1, 2], f32, name="warm", tag="warm")
        nc.gpsimd.memset(warm[:], 0.0)
        warm2 = misc.tile([1, 2], f32, name="warm2", tag="warm2")
        nc.scalar.activation(warm2[:], warm[:], AF.Sigmoid)

        # zeroed pads
        nc.gpsimd.memset(h1t[:, 0:B], 0.0)
        nc.gpsimd.memset(h2t[:, 0:B], 0.0)
        nc.gpsimd.memset(h2t[:, 17 * B : 18 * B], 0.0)

        # single psum layout for the whole kernel: 2x [128,1536] (3 banks
        # each) + 2x [128,512] pool/head psums = exactly 8 banks, so no
        # mid-kernel pool-close barrier is ever needed
        cps = stk.enter_context(tc.tile_pool(name="cps", bufs=2, space="PSUM"))
        smps = stk.enter_context(tc.tile_pool(name="smps", bufs=2, space="PSUM"))

        relu_cnt = [0]

        def relu_wide(dst, src, bias_nm):
            """relu(+bias) alternating ACT/DVE; bias layout must be uniform
            across the free dim (per-partition)."""
            use_act = relu_cnt[0] % 2 == 0
            relu_cnt[0] += 1
            if bias_nm is None:
                if use_act:
                    nc.scalar.activation(dst, src, AF.Relu)
                else:
                    nc.vector.tensor_scalar_max(dst, src, 0.0)
            else:
                bias = WF(bias_nm).bitcast(f32)
                if use_act:
                    nc.scalar.activation(dst, src, AF.Relu, bias=bias)
                else:
                    nc.vector.tensor_scalar(
                        dst, src, bias, 0.0,
                        mybir.AluOpType.add, mybir.AluOpType.max,
                    )

        def relu_both(dst_tile, dst_off, ps, w, bias_nm):
            """relu a [128,w] psum group: ACT takes the first 1024 cols
            (2 psum regions), DVE the remainder (region-aligned so each
            piece gates on an earlier matmul stop)."""
            bias = None if bias_nm is None else WF(bias_nm).bitcast(f32)
            cut = min(1024, w - 512) if w > 512 else w
            if bias is None:
                nc.scalar.activation(
                    dst_tile[:, dst_off : dst_off + cut], ps[:, 0:cut], AF.Relu
                )
            else:
                nc.scalar.activation(
                    dst_tile[:, dst_off : dst_off + cut], ps[:, 0:cut],
                    AF.Relu, bias=bias,
                )
            if cut < w:
                if bias is None:
                    nc.vector.tensor_scalar_max(
                        dst_tile[:, dst_off + cut : dst_off + w], ps[:, cut:w],
                        0.0,
                    )
                else:
                    nc.vector.tensor_scalar(
                        dst_tile[:, dst_off + cut : dst_off + w],
                        ps[:, cut:w], bias, 0.0,
                        mybir.AluOpType.add, mybir.AluOpType.max,
                    )

        # ---- conv1: 3-row groups (2 DR matmuls per row) ----
        c1_groups = [(0 + 3 * g, 3) for g in range(10)] + [(30, 2)]
        for y0, nr in c1_groups:
            ps = cps.tile([128, 1536], f32, name=f"c1ps{y0}", tag="cps")
            for j in range(nr):
                nc.tensor.matmul(
                    ps[:, j * 512 : (j + 1) * 512], DRW("c1A"), xsl(y0 + j),
                    start=True, stop=False, perf_mode=DRM,
                )
            for j in range(nr):
                nc.tensor.matmul(
                    ps[:, j * 512 : (j + 1) * 512], DRW("c1B"), xsl(y0 + j + 1),
                    start=False, stop=True, perf_mode=DRM,
                )
            relu_both(h1t, (y0 + 1) * B, ps, nr * 512, "bias1")

        # ---- conv2: 3-row groups ----
        c2_groups = [(3 * g, 3) for g in range(5)] + [(15, 1)]
        for y0, nr in c2_groups:
            ps = cps.tile([128, 1536], f32, name=f"c2ps{y0}", tag="cps")
            for j in range(nr):
                nc.tensor.matmul(
                    ps[:, j * 512 : (j + 1) * 512], DRW("c2A"),
                    h1sl(2 * (y0 + j)),
                    start=True, stop=False, perf_mode=DRM,
                )
            for j in range(nr):
                nc.tensor.matmul(
                    ps[:, j * 512 : (j + 1) * 512], DRW("c2B"),
                    h1sl(2 * (y0 + j) + 1),
                    start=False, stop=True, perf_mode=DRM,
                )
            relu_both(h2t, (y0 + 1) * B, ps, nr * 512, "bias2")

        plt = [
            smps.tile([128, 512], f32, name=f"plt{h}", tag="sm") for h in range(2)
        ]

        def _pool_pair(k):
            # stream k contracts h3 rows (k, k+8): slot0 via pm_yb0 -> out
            # partitions 0:16, slot1 via pm_yb1 -> partitions 16:32
            for half in range(2):
                nc.tensor.matmul(
                    plt[half][0:32, :],
                    W8("plAB").rearrange("p (s m) -> p s m", s=2),
                    h3v[:, k : k + 9 : 8, half, :],
                    start=(k == 0), stop=(k == 7), perf_mode=DRM,
                )

        # ---- conv3: 3-unit groups (unit = one (row, half), 2 DR each),
        # pool DR streams interleaved once both their h3 rows exist ----
        units = [(y, h) for y in range(16) for h in range(2)]
        ugroups = [units[i : i + 3] for i in range(0, 32, 3)]
        pool_emitted = [False] * 8
        for gi, grp in enumerate(ugroups):
            ps = cps.tile([128, 1536], f32, name=f"c3ps{gi}", tag="cps")
            for j, (y, h) in enumerate(grp):
                o = ps[:, j * 512 : (j + 1) * 512]
                nc.tensor.matmul(o, DRW(f"c3A{h}"), h2sl(y), start=True,
                                 stop=False, perf_mode=DRM)
                nc.tensor.matmul(o, DRW(f"c3B{h}"), h2sl(y + 1), start=False,
                                 stop=True, perf_mode=DRM)
            y0, h0 = grp[0]
            if zero_b3:
                relu_both(h3t, y0 * 1024 + h0 * 512, ps, len(grp) * 512, None)
            else:
                for j, (y, h) in enumerate(grp):
                    relu_wide(
                        h3v[:, y, h, :], ps[:, j * 512 : (j + 1) * 512],
                        f"bias3h{h}",
                    )
            # pool stream k reads h3 rows (k, k+8): ready one group after
            # unit 2(k+8)+1 has been emitted
            for k in range(8):
                if not pool_emitted[k] and (2 * k + 17) // 3 + 1 == gi:
                    _pool_pair(k)
                    pool_emitted[k] = True
        for k in range(8):
            if not pool_emitted[k]:
                _pool_pair(k)

        # ---- head ----
        hsb = stk.enter_context(tc.tile_pool(name="hsb", bufs=2))

        pooled = []
        for half in range(2):
            t = hsb.tile([32, B], fp16, name=f"pooled{half}", tag="pooled")
            if half == 0:
                nc.scalar.activation(t[:], plt[half][0:32, :], AF.Copy)
            else:
                nc.vector.tensor_copy(t[:], plt[half][0:32, :])
            pooled.append(t)

        # fc: feats = relu(wf @ pooled + bf) as two [128,B] chunks
        feats, sqf = [], []
        for mh in range(2):
            ps = cps.tile([128, 1536], f32, name=f"fcps{mh}", tag="cps")
            for half in range(2):
                nc.tensor.matmul(
                    ps[:, 0:512],
                    W16(f"wfh{half}")[:, mh * 128 : (mh + 1) * 128],
                    pooled[half][:],
                    start=(half == 0), stop=(half == 1),
                )
            f = hsb.tile([128, B], fp16, name=f"feats{mh}", tag="feats")
            bias = WF(f"bf{mh}").bitcast(f32)
            if mh == 0:
                nc.scalar.activation(f[:], ps[:, 0:512], AF.Relu, bias=bias)
            else:
                nc.vector.tensor_scalar(
                    f[:], ps[:, 0:512], bias, 0.0,
                    mybir.AluOpType.add, mybir.AluOpType.max,
                )
            feats.append(f)
        # squares of feats for |feats|^2
        for mh in range(2):
            s = hsb.tile([128, B], fp16, name=f"sqf{mh}", tag="sqf")
            if mh == 0:
                nc.scalar.activation(s[:], feats[mh][:], AF.Square)
            else:
                nc.vector.tensor_mul(s[:], feats[mh][:], feats[mh][:])
            sqf.append(s)
        ssps = smps.tile([128, 512], f32, name="ssps", tag="sm")
        for mh in range(2):
            nc.tensor.matmul(
                ssps[0:1, :], W16("ones")[:, 0:1], sqf[mh][:],
                start=(mh == 0), stop=(mh == 1),
            )
        # y = U @ feats; zsum = sum z_j y_j^2
        zsps = smps.tile([128, 512], f32, name="zsps", tag="sm")
        for mh in range(2):
            ups = cps.tile([128, 1536], f32, name=f"ups{mh}", tag="cps")
            for kc in range(2):
                nc.tensor.matmul(
                    ups[:, 0:512], W16(f"ut{kc}{mh}"), feats[kc][:],
                    start=(kc == 0), stop=(kc == 1),
                )
            sqy = hsb.tile([128, B], fp16, name=f"sqy{mh}", tag="sqy")
            nc.scalar.activation(sqy[:], ups[:, 0:512], AF.Square)
            nc.tensor.matmul(
                zsps[0:1, :],
                (W16("ones") if mh == 0 else W16("negones"))[:, 0:1],
                sqy[:],
                start=(mh == 0), stop=(mh == 1),
            )
        # q = zsum / ss  (ss = |feats|^2 is O(1)-bounded away from 0 here)
        rss = hsb.tile([1, B], f32, name="rss", tag="qrow", bufs=6)
        rscr = hsb.tile([1, B], f32, name="rscr", tag="qrow", bufs=6)
        nc.vector.reciprocal_approx_accurate(rss[:], ssps[0:1, :], rscr[:])
        q = hsb.tile([1, B], fp16, name="q", tag="qrow", bufs=6)
        nc.vector.tensor_mul(q[:], zsps[0:1, :], rss[:])

        # MLP collapsed to its exact piecewise-linear form:
        # logit = c0 + c1*q + sum_j a_j * relu(q - t_j)
        o = hsb.tile([1, B], f32, name="sb_o", tag="mlpo")
        ops = smps.tile([128, 512], f32, name="ps_o", tag="sm")
        if nk == 0:
            nc.tensor.matmul(ops[0:1, :], W16("pwl_c1"), q[:],
                             start=True, stop=True)
        elif nk == 1:
            r = hsb.tile([1, B], fp16, name="pwl_r", tag="mlpa")
            nc.scalar.activation(
                r[:], q[:], AF.Relu, bias=WF("pwl_negt")[0:1, :].bitcast(f32)
            )
            nc.tensor.matmul(ops[0:1, :], W16("pwl_a")[0:1, :], r[:],
                             start=True, stop=False)
            nc.tensor.matmul(ops[0:1, :], W16("pwl_c1"), q[:],
                             start=False, stop=True)
        else:
            kn = smps.tile([128, 512], f32, name="ps_kn", tag="sm")
            nc.tensor.matmul(kn[0:nk, :], W16("pwl_ones")[:, 0:nk], q[:],
                             start=True, stop=True)
            r = hsb.tile([nk, B], fp16, name="pwl_r", tag="mlpa")
            nc.scalar.activation(
                r[:], kn[0:nk, :], AF.Relu,
                bias=WF("pwl_negt")[0:nk, :].bitcast(f32),
            )
            nc.tensor.matmul(ops[0:1, :], W16("pwl_a")[0:nk, :], r[:],
                             start=True, stop=False)
            nc.tensor.matmul(ops[0:1, :], W16("pwl_c1"), q[:],
                             start=False, stop=True)
        nc.scalar.activation(
            o[:], ops[0:1, :], AF.Sigmoid, bias=WF("pwl_c0")[0:1, :].bitcast(f32)
        )

        nc.sync.dma_start(out_d[:], o[:])
        stk.close()

    nc.compile()
    return nc


def kernel(**inputs):
    from concourse import bass_utils

    x_slices, blob8, blob16, blobf, key = _host_prep(inputs)
    if key not in _COMPILED:
        _COMPILED[key] = _build_module(key)
    nc = _COMPILED[key]

    in_maps = [
        {"xq": x_slices[c], "wq8": blob8, "w16": blob16, "wf32": blobf}
        for c in range(N_CORES)
    ]
    res = bass_utils.run_bass_kernel_spmd(nc, in_maps, list(range(N_CORES)))
    outs = [res.results[c]["out"].reshape(B, 1) for c in range(N_CORES)]
    return np.concatenate(outs, axis=0).astype(np.float32)
